# revision 37
# baseline (speedup 1.0000x reference)
"""Trainium2 Bass kernel for nn_Attention_18863496364032 (self-contained).

fused attention block: qkv proj -> 16-head scaled-dot-product attention ->
out proj + bias -> LayerNorm, for x [4, 2048, 1024] f32.

Sharded over 8 NeuronCores with zero cross-core communication: core c
handles batch b = c//2 and query-token half r = c%2 (1024 query rows),
recomputing K/V for its batch locally. Inputs are transposed/cast to bf16
on the host (layout prep only); all matmuls/softmax/layernorm run on-core
(bf16 operands, fp32 accumulation).

Scheduling notes (why it looks the way it does):
- The softmax exp stream on ScalarE (~1.15us per 128-key chunk) is nearly
  co-limiting with the PE; anything else placed on ScalarE mid-round
  stalls the PV pipeline. So attention rounds store UNNORMALIZED ao plus
  per-head denominators (den_all), and the 1/den reciprocals run as two
  wide [heads, TQ] ACT ops in the final round's tail where ScalarE idles.
- Reciprocal rows are broadcast across partitions by a DRAM round-trip
  (stride-0 partition DMA) instead of engine ops, then aoT is normalized
  by 8 DVE mults that overlap the out-projection.
- s1 (x / qkv weight tiles) is released at the last attention round so
  the stage-3 tiles and w_out prefetch can start early.
"""

import numpy as np
import ml_dtypes

import concourse.bass as bass
import concourse.mybir as mybir
import concourse.tile as tile
from concourse.bass_utils import run_bass_kernel_spmd
from concourse.vector_clock import ScopedClock

BF16 = mybir.dt.bfloat16
F32 = mybir.dt.float32
AF = mybir.ActivationFunctionType
ALU = mybir.AluOpType

# ---------------------------------------------------------------------------
# Workarounds for the container toolchain (walrus rejects >1 sync-wait per
# instruction; the Tile end-of-kernel drain carries several).
# ---------------------------------------------------------------------------


def _drain_and_barrier_split(self, tick_clock, wait_clock):
    nc = self.nc
    probe = nc.sync.nop()
    wait_clock.add_sem_waits(probe.ins, ScopedClock({None: tick_clock.global_clock}))
    si = probe.ins.sync_info
    waits = list(si.on_wait) if si is not None and si.on_wait else []
    if len(waits) > 1:
        probe.ins.sync_info = mybir.SyncInfo(on_wait=waits[:1], on_update=[])
        for w in waits[1:]:
            extra = nc.sync.nop()
            extra.ins.sync_info = mybir.SyncInfo(on_wait=[w], on_update=[])
    nc.sync.drain()

    nc.all_engine_barrier()
    assert self.sems is not None
    popped = nc._tile_sem_poison_stack.pop()
    assert popped is self._sem_poison
    nc.clear_and_free_semaphores(list(self.sems.allocated().values()))
    nc.all_engine_barrier()


tile.TileContext._drain_and_barrier = _drain_and_barrier_split

_nsplit = [0]


def split_excess_waits(nc, max_waits=1):
    """Hoist excess sync waits onto same-engine nops placed before."""
    n = 0
    for f in nc.m.functions:
        for blk in f.blocks:
            out = []
            changed = False
            for inst in blk.instructions:
                si = inst.sync_info
                waits = list(si.on_wait) if si is not None and si.on_wait else []
                if len(waits) > max_waits:
                    changed = True
                    extra, keep = waits[:-max_waits], waits[-max_waits:]
                    for i in range(0, len(extra), max_waits):
                        _nsplit[0] += 1
                        n += 1
                        nop = mybir.InstNoOp(
                            name=f"I-waitsplit-{_nsplit[0]}", ins=[], outs=[])
                        nop.engine = inst.engine
                        nop.sync_info = mybir.SyncInfo(
                            on_wait=extra[i:i + max_waits], on_update=[])
                        out.append(nop)
                    inst.sync_info = mybir.SyncInfo(
                        on_wait=keep,
                        on_update=list(si.on_update) if si.on_update else [])
                out.append(inst)
            if changed:
                blk.instructions = out
    return n


# ---------------------------------------------------------------------------
# Kernel builder
# ---------------------------------------------------------------------------


import numpy as np
import ml_dtypes

import concourse.bass as bass
import concourse.mybir as mybir
import concourse.tile as tile

BF16 = mybir.dt.bfloat16
F32 = mybir.dt.float32
AF = mybir.ActivationFunctionType
ALU = mybir.AluOpType


def _bcast_ap(ap, p=128):
    # replicate a [N] dram tensor across p partitions during DMA
    return bass.AP(tensor=ap.tensor, offset=ap.offset, ap=[[0, p]] + list(ap.ap))


def build_nc(DIM=1024, TKV=2048, TQ=1024, H=16, EPS=1e-5):
    D = 64
    INNER = H * D
    HP = H // 2            # head pairs == c-tiles of q/k
    DT = DIM // 128        # contraction tiles over model dim
    CT = INNER // 128      # c-tiles of q/k/v/ao
    TT = TKV // 128        # key/value token tiles
    NQT = TQ // 128        # output token tiles
    assert CT == HP
    scale = float(D) ** -0.5

    def fblocks(total, blk=512):
        return [(i, min(blk, total - i)) for i in range(0, total, blk)]

    IH = fblocks(TQ, 512)   # query half-blocks
    NIH = len(IH)

    nc = bass.Bass()
    xkv_d = nc.declare_dram_parameter("xT_kv", [DIM, TKV], BF16, isOutput=False)
    xq_d = nc.declare_dram_parameter("xT_q", [DIM, TQ], BF16, isOutput=False)
    wq_d = nc.declare_dram_parameter("w_q", [DIM, INNER], BF16, isOutput=False)
    wk_d = nc.declare_dram_parameter("w_k", [DIM, INNER], BF16, isOutput=False)
    wv_d = nc.declare_dram_parameter("w_v", [DIM, INNER], BF16, isOutput=False)
    wo_d = nc.declare_dram_parameter("w_out", [INNER, DIM], BF16, isOutput=False)
    bo_d = nc.declare_dram_parameter("b_out", [1, DIM], BF16, isOutput=False)
    g_d = nc.declare_dram_parameter("ln_gamma", [DIM], F32, isOutput=False)
    be_d = nc.declare_dram_parameter("ln_beta", [DIM], F32, isOutput=False)
    y_d = nc.declare_dram_parameter("y", [TQ, DIM], F32, isOutput=True)

    with tile.TileContext(nc) as tc:
        with (
            tc.tile_pool(name="consts", bufs=1) as consts,
            tc.tile_pool(name="persist", bufs=1) as persist,
            tc.tile_pool(name="kqrot", bufs=2) as kqrot,
            tc.tile_pool(name="ps_s", bufs=3, space="PSUM") as ps_s,
            tc.tile_pool(name="ppool", bufs=7) as ppool,
            tc.tile_pool(name="work", bufs=4) as work,
        ):
            ps_ao = tc.alloc_tile_pool(name="ps_ao", bufs=2, space="PSUM")
            ones_row = consts.tile([1, 128], BF16, tag="ones_row", name="ones_row")
            nc.vector.memset(ones_row, 1.0)
            gamma_b = consts.tile([128, DIM], F32, tag="gamma", name="gamma")
            nc.sync.dma_start(out=gamma_b, in_=_bcast_ap(g_d[:]))
            beta_b = consts.tile([128, DIM], F32, tag="beta", name="beta")
            nc.sync.dma_start(out=beta_b, in_=_bcast_ap(be_d[:]))
            bo_sb = consts.tile([1, DIM], BF16, tag="bo", name="bo")
            nc.sync.dma_start(out=bo_sb, in_=bo_d[:])
            eps_sb = consts.tile([128, 1], F32, tag="eps", name="eps")
            nc.vector.memset(eps_sb, EPS)

            # v_aug layout: per head 65 columns = [v_h (64) | ones]
            v_t = [persist.tile([128, H * 65], BF16, tag=f"v{t}", name=f"v{t}")
                   for t in range(TT)]
            aoT = [persist.tile([128, TQ], BF16, tag=f"aoT{c}", name=f"aoT{c}")
                   for c in range(CT)]

            s1 = tc.alloc_tile_pool(name="s1", bufs=1)
            xkv = [s1.tile([128, TKV], BF16, tag=f"xkv{d}", name=f"xkv{d}")
                   for d in range(DT)]
            xq = [s1.tile([128, TQ], BF16, tag=f"xq{d}", name=f"xq{d}")
                  for d in range(DT)]
            wq = [s1.tile([128, INNER], BF16, tag=f"wq{d}", name=f"wq{d}")
                  for d in range(DT)]
            wk = [s1.tile([128, INNER], BF16, tag=f"wk{d}", name=f"wk{d}")
                  for d in range(DT)]
            wv = [s1.tile([128, INNER], BF16, tag=f"wv{d}", name=f"wv{d}")
                  for d in range(DT)]
            # DMA in dependency order: the slices the upfront tasks read
            # land first (v[0..1] needs xkv + wv cols 0:512; kT0/qT0 block 0
            # needs wk/wq cols 0:128 and xq cols 0:512), the rest streams in
            # behind while round 0 is already running.
            h1 = min(512, INNER)
            hq = min(512, TQ)
            for d in range(DT):
                r = slice(d * 128, (d + 1) * 128)
                nc.sync.dma_start(out=xkv[d], in_=xkv_d[r, :])
                nc.sync.dma_start(out=wv[d][:, 0:h1], in_=wv_d[r, 0:h1])
            for d in range(DT):
                r = slice(d * 128, (d + 1) * 128)
                nc.sync.dma_start(out=wk[d][:, 0:128], in_=wk_d[r, 0:128])
                nc.sync.dma_start(out=xq[d][:, 0:hq], in_=xq_d[r, 0:hq])
                nc.sync.dma_start(out=wq[d][:, 0:128], in_=wq_d[r, 0:128])
            for d in range(DT):
                r = slice(d * 128, (d + 1) * 128)
                if h1 < INNER:
                    nc.sync.dma_start(out=wv[d][:, h1:], in_=wv_d[r, h1:])
                nc.sync.dma_start(out=wk[d][:, 128:], in_=wk_d[r, 128:])
                if hq < TQ:
                    nc.sync.dma_start(out=xq[d][:, hq:], in_=xq_d[r, hq:])
                nc.sync.dma_start(out=wq[d][:, 128:], in_=wq_d[r, 128:])

            # ---------- production tasks ----------
            def v_task(t, off, w):
                def run():
                    ps = ps_s.tile([128, 1024], F32, tag="sp", name="prv")
                    for d in range(DT):
                        nc.tensor.matmul(
                            ps[:, :w],
                            lhsT=xkv[d][:, t * 128:(t + 1) * 128],
                            rhs=wv[d][:, off:off + w],
                            start=(d == 0), stop=(d == DT - 1),
                        )
                    h0, nh = off // 64, w // 64
                    dst = v_t[t].rearrange("p (h e) -> p h e", e=65)
                    nc.vector.tensor_copy(
                        dst[:, h0:h0 + nh, 0:64],
                        ps[:, :w].rearrange("p (h e) -> p h e", e=64),
                    )
                    nc.vector.memset(dst[:, h0:h0 + nh, 64:65], 1.0)
                return run

            def kq_task(kt, c, tb, w, wsrc, xsrc):
                def run():
                    ps = ps_s.tile([128, 1024], F32, tag="sp", name="prk")
                    for d in range(DT):
                        nc.tensor.matmul(
                            ps[:, :w],
                            lhsT=wsrc[d][:, c * 128:(c + 1) * 128],
                            rhs=xsrc[d][:, tb:tb + w],
                            start=(d == 0), stop=(d == DT - 1),
                        )
                    nc.vector.tensor_copy(kt[:, tb:tb + w], ps[:, :w])
                return run

            def kq_tasks(c):
                kt = kqrot.tile([128, TKV], BF16, tag="kT", name=f"kT{c}")
                qt = kqrot.tile([128, TQ], BF16, tag="qT", name=f"qT{c}")
                tasks = [kq_task(kt, c, tb, w, wk, xkv) for tb, w in fblocks(TKV)]
                tasks += [kq_task(qt, c, tb, w, wq, xq) for tb, w in fblocks(TQ)]
                return kt, qt, tasks

            # PE warmup: dependency-free matmuls on constant tiles run
            # during the initial DMA wave and un-throttle the HAM clock
            # before the first real production matmuls.
            wps = ps_s.tile([128, 1024], F32, tag="sp", name="warm")
            for _ in range(56):
                nc.tensor.matmul(
                    wps[0:128, 0:128], lhsT=ones_row, rhs=ones_row[:, 0:128],
                    start=True, stop=True,
                )
            nc.vector.tensor_copy(
                work.tile([128, 128], F32, tag="wsink", name="wsink"),
                wps[:, 0:128])

            # v columns are produced just-in-time by half: heads 0..H/2-1
            # (first 512-block) feed rounds 0..HP/2-1, the rest feed later
            # rounds. v[0],v[1] (first half) plus pair 0's kT/qT up front.
            VHALF = fblocks(INNER)
            # six v tiles up front: this PE work runs while the second DMA
            # wave streams, and round 0's per-chunk task load stays light
            # enough that ScalarE never starves.
            NVUP = min(8, TT)
            for t in range(NVUP):
                v_task(t, *VHALF[0])()
            kt_cur, qt_cur, tasks0 = kq_tasks(0)
            nkt = len(fblocks(TKV))
            # only the first kT/qT blocks up front; the rest just-in-time in
            # round 0 (block b of kT is first read at chunk jt = 4*b)
            rest0 = tasks0[1:nkt] + tasks0[nkt + 1:]
            tasks0[0]()
            tasks0[nkt]()

            pending = []
            # second-half v tiles (and the whole tail of VHALF splits) spread
            # across rounds 1..HP/2; they are only read from round HP/2 on.
            spares = []
            for off, w in VHALF[1:]:
                spares += [v_task(t, off, w) for t in range(TT)]

            # aoT stays UNNORMALIZED through the attention rounds; the
            # softmax denominators are stashed per head in den_all and the
            # normalization happens in stage 3 (one wide ACT reciprocal +
            # DMA broadcast + DVE mults, all off the round boundaries).
            den_all = persist.tile([2 * HP, TQ], F32, tag="den_all",
                                   name="den_all")

            def copy_piece(hp, sub, aot, ioff, iw):
                # engines can only address 32-aligned partition bases, so the
                # den row goes through a base-0 staging tile + SBUF-SBUF DMA.
                def run():
                    nc.vector.tensor_copy(
                        aoT[hp][sub * 64:sub * 64 + 64, ioff:ioff + iw],
                        aot[0:64, 0:iw])
                    stg = work.tile([1, 512], F32, tag="dstg", name="dstg")
                    nc.vector.tensor_copy(stg[:, :iw], aot[64:65, 0:iw])
                    h = 2 * hp + sub
                    nc.sync.dma_start(
                        out=den_all[h:h + 1, ioff:ioff + iw],
                        in_=stg[:, :iw])
                return run

            deferred = []
            late = None
            dramp = tc.alloc_tile_pool(name="dramp", bufs=1, space="DRAM")
            for hp in range(HP):
                if hp + 1 == HP:
                    # s1 (x / qkv weights) is dead once hp=7's kT/qT exist;
                    # reclaim it early so stage-3 tiles + wo prefetch can
                    # start during the last round.
                    s1.release()
                    late = tc.alloc_tile_pool(name="late", bufs=1)
                    wo = [late.tile([128, DIM], BF16, tag=f"wo{c}",
                                    name=f"wo{c}") for c in range(CT)]
                    for c in range(CT):
                        nc.sync.dma_start(
                            out=wo[c], in_=wo_d[c * 128:(c + 1) * 128, :])
                    recl = late.tile([2 * HP, TQ], F32, tag="recl",
                                     name="recl")
                    recb = late.tile([2 * HP, TQ], BF16, tag="recb16",
                                     name="recb16")
                    recd = dramp.tile([2 * HP, TQ], BF16, tag="recd",
                                      name="recd")
                    rbt = [late.tile([128, TQ], BF16, tag=f"rbt{c}",
                                     name=f"rbt{c}") for c in range(CT)]
                    aoTn = [late.tile([128, TQ], BF16, tag=f"an{c}",
                                      name=f"an{c}") for c in range(CT)]

                    def den_batch(rlo, rhi, clo, chi):
                        # reciprocal of den rows [rlo:rhi] + rb broadcast and
                        # aoT normalize for c-tiles [clo:chi]. ACT ops always
                        # start at partition 0 (engine partition alignment);
                        # only the DMA slices the fresh rows.
                        def run():
                            nc.scalar.activation(recl[0:rhi],
                                                 den_all[0:rhi], AF.Ln)
                            nc.scalar.activation(recb[0:rhi],
                                                 recl[0:rhi],
                                                 AF.Exp, scale=-1.0)
                            nc.sync.dma_start(out=recd[rlo:rhi, :],
                                              in_=recb[rlo:rhi])
                            for c in range(clo, chi):
                                for half in range(2):
                                    rsl = recd[2 * c + half:
                                               2 * c + half + 1, :]
                                    src = bass.AP(
                                        tensor=rsl.tensor, offset=rsl.offset,
                                        ap=[[0, 64]] + list(rsl.ap)[1:])
                                    nc.sync.dma_start(
                                        out=rbt[c][64 * half:64 * half + 64,
                                                   :],
                                        in_=src)
                                nc.vector.tensor_mul(aoTn[c], aoT[c], rbt[c])
                        return run
                if hp + 1 < HP:
                    kt_nxt, qt_nxt, nxt = kq_tasks(hp + 1)
                    pending.extend(nxt)
                if 1 <= hp < max(2, HP // 2):
                    nsp = (len(spares) + max(2, HP // 2) - 1 - hp) // (
                        max(2, HP // 2) - hp)
                    pending.extend(spares[:nsp])
                    spares = spares[nsp:]
                h0, h1 = 2 * hp, 2 * hp + 1
                for i, (ioff, iw) in enumerate(IH):
                    ao0 = ps_ao.tile([65, 512], F32, tag="ao", name="ao")
                    ao1 = ps_ao.tile([65, 512], F32, tag="ao", name="ao")
                    # software pipeline: PV consumes p several chunks behind
                    # the scores/exp front so the PE never waits on a fresh
                    # exp (and the round-boundary norm ACT ops hide too).
                    LAG = 5
                    pq = []
                    for jt in range(TT + LAG):
                        if jt < TT:
                            # scores+exp first: ScalarE gets its input at
                            # chunk start, production tasks follow in its
                            # shadow
                            sp = ps_s.tile([128, 1024], F32, tag="sp", name="sp")
                            nc.tensor.matmul(
                                sp[:, 0:iw],
                                lhsT=kt_cur[0:64, jt * 128:(jt + 1) * 128],
                                rhs=qt_cur[0:64, ioff:ioff + iw],
                                start=True, stop=True, tile_position=(0, 0),
                            )
                            nc.tensor.matmul(
                                sp[:, iw:2 * iw],
                                lhsT=kt_cur[64:128, jt * 128:(jt + 1) * 128],
                                rhs=qt_cur[64:128, ioff:ioff + iw],
                                start=True, stop=True, tile_position=(64, 0),
                            )
                            p = ppool.tile([128, 1024], BF16, tag="p", name="p")
                            nc.scalar.activation(p[:, 0:2 * iw], sp[:, 0:2 * iw],
                                                 AF.Exp, scale=scale)
                            pq.append(p)
                            if hp == 0 and i == 0:
                                # dense early pops: kT block b is first read
                                # at chunk 4*b, and each kq block takes ~2.5us
                                # to produce — jt in (0,2,4,8) left scores
                                # waiting on fresh kT through chunks 4-12
                                if jt < 4 and rest0:
                                    rest0.pop(0)()
                                if jt + NVUP < TT:
                                    v_task(jt + NVUP, *VHALF[0])()
                            elif jt % 3 == 1 and pending:
                                pending.pop(0)()
                        if jt >= LAG:
                            pjt = jt - LAG
                            p = pq.pop(0)
                            first, last = (pjt == 0), (pjt == TT - 1)
                            nc.tensor.matmul(
                                ao0[0:65, 0:iw],
                                lhsT=v_t[pjt][:, h0 * 65:h0 * 65 + 65],
                                rhs=p[:, 0:iw],
                                start=first, stop=last,
                            )
                            nc.tensor.matmul(
                                ao1[0:65, 0:iw],
                                lhsT=v_t[pjt][:, h1 * 65:h1 * 65 + 65],
                                rhs=p[:, iw:2 * iw],
                                start=first, stop=last,
                            )
                        if jt in (0, 3) and deferred:
                            deferred.pop(0)()
                        if hp + 1 == HP and i == 1 and jt == TT:
                            # ScalarE goes idle in the final round's tail:
                            # run the reciprocal chain for heads 0..13 here.
                            den_batch(0, 2 * HP - 2, 0, CT - 1)()
                    deferred.append(copy_piece(hp, 0, ao0, ioff, iw))
                    deferred.append(copy_piece(hp, 1, ao1, ioff, iw))
                if hp + 1 < HP:
                    while pending:
                        pending.pop(0)()
                    kt_cur, qt_cur = kt_nxt, qt_nxt
            while deferred:
                deferred.pop(0)()
            # last two heads' reciprocal chain (their den rows finalize only
            # after the final round's copies)
            den_batch(2 * HP - 2, 2 * HP, CT - 1, CT)()
            ps_ao.release()

            # ---- stage 3: out-projection + bias + LayerNorm ----
            yout = tc.alloc_tile_pool(name="yout", bufs=2)
            for t in range(NQT):
                yps = ps_s.tile([128, DIM], F32, tag="sp", name="yps")
                for c in range(CT):
                    for off, w in fblocks(DIM):
                        nc.tensor.matmul(
                            yps[:, off:off + w],
                            lhsT=aoTn[c][:, t * 128:(t + 1) * 128],
                            rhs=wo[c][:, off:off + w],
                            start=(c == 0), stop=False,
                        )
                for off, w in fblocks(DIM):
                    nc.tensor.matmul(
                        yps[:, off:off + w],
                        lhsT=ones_row,
                        rhs=bo_sb[:, off:off + w],
                        start=False, stop=True,
                    )
                ng = (DIM + 511) // 512
                st = work.tile([128, ng, 6], F32, tag="bnst", name="bnst")
                for g in range(ng):
                    gw = min(512, DIM - g * 512)
                    nc.vector.bn_stats(st[:, g, :], yps[:, g * 512:g * 512 + gw])
                mv = work.tile([128, 2], F32, tag="mv", name="mv")
                nc.vector.bn_aggr(mv, st)
                # rstd = sqrt(1/(var+eps)): DVE reciprocal + ACT sqrt keeps
                # the heavy per-element work off ScalarE.
                ve = work.tile([128, 1], F32, tag="lnv", name="ve")
                nc.vector.tensor_scalar(ve, mv[:, 1:2], scalar1=EPS,
                                        scalar2=None, op0=ALU.add)
                vr = work.tile([128, 1], F32, tag="vr", name="vr")
                nc.vector.reciprocal(vr, ve)
                rstd = work.tile([128, 1], F32, tag="rstd", name="rstd")
                nc.scalar.activation(rstd, vr, AF.Sqrt)
                nmr = work.tile([128, 1], F32, tag="nmr", name="nmr")
                nc.vector.tensor_scalar(
                    nmr, mv[:, 0:1], scalar1=rstd, scalar2=-1.0,
                    op0=ALU.mult, op1=ALU.mult,
                )
                # (y - mu)*rstd on ScalarE (idle in this phase): Identity
                # with per-partition scale/bias APs.
                yA = yout.tile([128, DIM], F32, tag="yA", name="yA")
                nc.scalar.activation(yA, yps, AF.Identity, bias=nmr,
                                     scale=rstd)
                yn = yout.tile([128, DIM], F32, tag="yn", name="yn")
                nc.vector.tensor_mul(yn, yA, gamma_b)
                yn2 = yout.tile([128, DIM], F32, tag="yn2", name="yn2")
                # beta add on GpSimd mid-stage (spreads load), DVE for the
                # last blocks (GpSimd's 2.3us latency would extend the tail)
                eng = nc.gpsimd if t < NQT - 2 else nc.vector
                eng.tensor_add(yn2, yn, beta_b)
                nc.sync.dma_start(out=y_d[t * 128:(t + 1) * 128, :], in_=yn2)
            yout.release()
            late.release()

    return nc


def host_inputs(x_b, r, w_qkv, w_out, b_out, ln_gamma, ln_beta, TQ):
    """Build one core's input map from full fp32 arrays. x_b: [T, DIM]."""
    bf = ml_dtypes.bfloat16
    DIM = x_b.shape[1]
    INNER = w_qkv.shape[1] // 3
    xT = np.ascontiguousarray(x_b.T).astype(bf)
    xTq = np.ascontiguousarray(x_b[r * TQ:(r + 1) * TQ].T).astype(bf)
    return {
        "xT_kv": xT,
        "xT_q": xTq,
        "w_q": np.ascontiguousarray(w_qkv[:, :INNER]).astype(bf),
        "w_k": np.ascontiguousarray(w_qkv[:, INNER:2 * INNER]).astype(bf),
        "w_v": np.ascontiguousarray(w_qkv[:, 2 * INNER:]).astype(bf),
        "w_out": np.ascontiguousarray(w_out).astype(bf),
        "b_out": np.ascontiguousarray(b_out[None, :]).astype(bf),
        "ln_gamma": np.ascontiguousarray(ln_gamma).astype(np.float32),
        "ln_beta": np.ascontiguousarray(ln_beta).astype(np.float32),
    }


_NC_CACHE = {}


def _get_nc():
    if "nc" not in _NC_CACHE:
        nc = build_nc()
        split_excess_waits(nc)
        _NC_CACHE["nc"] = nc
    return _NC_CACHE["nc"]


def kernel(x, w_qkv, w_out, b_out, ln_gamma, ln_beta):
    x = np.asarray(x, dtype=np.float32)
    w_qkv = np.asarray(w_qkv, dtype=np.float32)
    w_out = np.asarray(w_out, dtype=np.float32)
    b_out = np.asarray(b_out, dtype=np.float32)
    ln_gamma = np.asarray(ln_gamma, dtype=np.float32)
    ln_beta = np.asarray(ln_beta, dtype=np.float32)
    B, N, DIM = x.shape
    TQ = N // 2
    nc = _get_nc()
    in_maps = []
    for c in range(8):
        b, r = c // 2, c % 2
        in_maps.append(host_inputs(
            x[b], r, w_qkv, w_out, b_out, ln_gamma, ln_beta, TQ))
    res = run_bass_kernel_spmd(nc, in_maps, list(range(8)))
    out = np.empty((B, N, DIM), np.float32)
    for c in range(8):
        b, r = c // 2, c % 2
        out[b, r * TQ:(r + 1) * TQ] = res.results[c]["y"]
    return out



# revision 38
# speedup vs baseline: 1.0152x; 1.0152x over previous
"""Trainium2 Bass kernel for nn_Attention_18863496364032 (self-contained).

fused attention block: qkv proj -> 16-head scaled-dot-product attention ->
out proj + bias -> LayerNorm, for x [4, 2048, 1024] f32.

Sharded over 8 NeuronCores with zero cross-core communication: core c
handles batch b = c//2 and query-token half r = c%2 (1024 query rows),
recomputing K/V for its batch locally. Inputs are transposed/cast to bf16
on the host (layout prep only); all matmuls/softmax/layernorm run on-core
(bf16 operands, fp32 accumulation).

Scheduling notes (why it looks the way it does):
- The softmax exp stream on ScalarE (~1.15us per 128-key chunk) is nearly
  co-limiting with the PE; anything else placed on ScalarE mid-round
  stalls the PV pipeline. So attention rounds store UNNORMALIZED ao plus
  per-head denominators (den_all), and the 1/den reciprocals run as two
  wide [heads, TQ] ACT ops in the final round's tail where ScalarE idles.
- Reciprocal rows are broadcast across partitions by a DRAM round-trip
  (stride-0 partition DMA) instead of engine ops, then aoT is normalized
  by 8 DVE mults that overlap the out-projection.
- s1 (x / qkv weight tiles) is released at the last attention round so
  the stage-3 tiles and w_out prefetch can start early.
"""

import numpy as np
import ml_dtypes

import concourse.bass as bass
import concourse.mybir as mybir
import concourse.tile as tile
from concourse.bass_utils import run_bass_kernel_spmd
from concourse.vector_clock import ScopedClock

BF16 = mybir.dt.bfloat16
F32 = mybir.dt.float32
AF = mybir.ActivationFunctionType
ALU = mybir.AluOpType

# ---------------------------------------------------------------------------
# Workarounds for the container toolchain (walrus rejects >1 sync-wait per
# instruction; the Tile end-of-kernel drain carries several).
# ---------------------------------------------------------------------------


def _drain_and_barrier_split(self, tick_clock, wait_clock):
    nc = self.nc
    probe = nc.sync.nop()
    wait_clock.add_sem_waits(probe.ins, ScopedClock({None: tick_clock.global_clock}))
    si = probe.ins.sync_info
    waits = list(si.on_wait) if si is not None and si.on_wait else []
    if len(waits) > 1:
        probe.ins.sync_info = mybir.SyncInfo(on_wait=waits[:1], on_update=[])
        for w in waits[1:]:
            extra = nc.sync.nop()
            extra.ins.sync_info = mybir.SyncInfo(on_wait=[w], on_update=[])
    nc.sync.drain()

    nc.all_engine_barrier()
    assert self.sems is not None
    popped = nc._tile_sem_poison_stack.pop()
    assert popped is self._sem_poison
    nc.clear_and_free_semaphores(list(self.sems.allocated().values()))
    nc.all_engine_barrier()


tile.TileContext._drain_and_barrier = _drain_and_barrier_split

_nsplit = [0]


def split_excess_waits(nc, max_waits=1):
    """Hoist excess sync waits onto same-engine nops placed before."""
    n = 0
    for f in nc.m.functions:
        for blk in f.blocks:
            out = []
            changed = False
            for inst in blk.instructions:
                si = inst.sync_info
                waits = list(si.on_wait) if si is not None and si.on_wait else []
                if len(waits) > max_waits:
                    changed = True
                    extra, keep = waits[:-max_waits], waits[-max_waits:]
                    for i in range(0, len(extra), max_waits):
                        _nsplit[0] += 1
                        n += 1
                        nop = mybir.InstNoOp(
                            name=f"I-waitsplit-{_nsplit[0]}", ins=[], outs=[])
                        nop.engine = inst.engine
                        nop.sync_info = mybir.SyncInfo(
                            on_wait=extra[i:i + max_waits], on_update=[])
                        out.append(nop)
                    inst.sync_info = mybir.SyncInfo(
                        on_wait=keep,
                        on_update=list(si.on_update) if si.on_update else [])
                out.append(inst)
            if changed:
                blk.instructions = out
    return n


# ---------------------------------------------------------------------------
# Kernel builder
# ---------------------------------------------------------------------------


import numpy as np
import ml_dtypes

import concourse.bass as bass
import concourse.mybir as mybir
import concourse.tile as tile

BF16 = mybir.dt.bfloat16
F32 = mybir.dt.float32
AF = mybir.ActivationFunctionType
ALU = mybir.AluOpType


def _bcast_ap(ap, p=128):
    # replicate a [N] dram tensor across p partitions during DMA
    return bass.AP(tensor=ap.tensor, offset=ap.offset, ap=[[0, p]] + list(ap.ap))


def build_nc(DIM=1024, TKV=2048, TQ=1024, H=16, EPS=1e-5):
    D = 64
    INNER = H * D
    HP = H // 2            # head pairs == c-tiles of q/k
    DT = DIM // 128        # contraction tiles over model dim
    CT = INNER // 128      # c-tiles of q/k/v/ao
    TT = TKV // 128        # key/value token tiles
    NQT = TQ // 128        # output token tiles
    assert CT == HP
    scale = float(D) ** -0.5

    def fblocks(total, blk=512):
        return [(i, min(blk, total - i)) for i in range(0, total, blk)]

    IH = fblocks(TQ, 512)   # query half-blocks
    NIH = len(IH)

    nc = bass.Bass()
    xkv_d = nc.declare_dram_parameter("xT_kv", [DIM, TKV], BF16, isOutput=False)
    xq_d = nc.declare_dram_parameter("xT_q", [DIM, TQ], BF16, isOutput=False)
    wq_d = nc.declare_dram_parameter("w_q", [DIM, INNER], BF16, isOutput=False)
    wk_d = nc.declare_dram_parameter("w_k", [DIM, INNER], BF16, isOutput=False)
    wv_d = nc.declare_dram_parameter("w_v", [DIM, INNER], BF16, isOutput=False)
    wo_d = nc.declare_dram_parameter("w_out", [INNER, DIM], BF16, isOutput=False)
    bo_d = nc.declare_dram_parameter("b_out", [1, DIM], BF16, isOutput=False)
    g_d = nc.declare_dram_parameter("ln_gamma", [DIM], F32, isOutput=False)
    be_d = nc.declare_dram_parameter("ln_beta", [DIM], F32, isOutput=False)
    y_d = nc.declare_dram_parameter("y", [TQ, DIM], F32, isOutput=True)

    with tile.TileContext(nc) as tc:
        with (
            tc.tile_pool(name="consts", bufs=1) as consts,
            tc.tile_pool(name="persist", bufs=1) as persist,
            tc.tile_pool(name="kqrot", bufs=2) as kqrot,
            tc.tile_pool(name="ps_s", bufs=3, space="PSUM") as ps_s,
            tc.tile_pool(name="ppool", bufs=7) as ppool,
            tc.tile_pool(name="work", bufs=4) as work,
        ):
            ps_ao = tc.alloc_tile_pool(name="ps_ao", bufs=2, space="PSUM")
            ones_row = consts.tile([1, 128], BF16, tag="ones_row", name="ones_row")
            nc.vector.memset(ones_row, 1.0)
            gamma_b = consts.tile([128, DIM], F32, tag="gamma", name="gamma")
            nc.sync.dma_start(out=gamma_b, in_=_bcast_ap(g_d[:]))
            beta_b = consts.tile([128, DIM], F32, tag="beta", name="beta")
            nc.sync.dma_start(out=beta_b, in_=_bcast_ap(be_d[:]))
            bo_sb = consts.tile([1, DIM], BF16, tag="bo", name="bo")
            nc.sync.dma_start(out=bo_sb, in_=bo_d[:])
            eps_sb = consts.tile([128, 1], F32, tag="eps", name="eps")
            nc.vector.memset(eps_sb, EPS)

            # v_aug layout: per head 65 columns = [v_h (64) | ones]
            v_t = [persist.tile([128, H * 65], BF16, tag=f"v{t}", name=f"v{t}")
                   for t in range(TT)]
            aoT = [persist.tile([128, TQ], BF16, tag=f"aoT{c}", name=f"aoT{c}")
                   for c in range(CT)]

            s1 = tc.alloc_tile_pool(name="s1", bufs=1)
            xkv = [s1.tile([128, TKV], BF16, tag=f"xkv{d}", name=f"xkv{d}")
                   for d in range(DT)]
            xq = [s1.tile([128, TQ], BF16, tag=f"xq{d}", name=f"xq{d}")
                  for d in range(DT)]
            wq = [s1.tile([128, INNER], BF16, tag=f"wq{d}", name=f"wq{d}")
                  for d in range(DT)]
            wk = [s1.tile([128, INNER], BF16, tag=f"wk{d}", name=f"wk{d}")
                  for d in range(DT)]
            wv = [s1.tile([128, INNER], BF16, tag=f"wv{d}", name=f"wv{d}")
                  for d in range(DT)]
            # DMA in dependency order: the slices the upfront tasks read
            # land first (v[0..1] needs xkv + wv cols 0:512; kT0/qT0 block 0
            # needs wk/wq cols 0:128 and xq cols 0:512), the rest streams in
            # behind while round 0 is already running.
            h1 = min(512, INNER)
            hq = min(512, TQ)
            for d in range(DT):
                r = slice(d * 128, (d + 1) * 128)
                nc.sync.dma_start(out=xkv[d], in_=xkv_d[r, :])
                nc.sync.dma_start(out=wv[d][:, 0:h1], in_=wv_d[r, 0:h1])
            for d in range(DT):
                r = slice(d * 128, (d + 1) * 128)
                nc.sync.dma_start(out=wk[d][:, 0:128], in_=wk_d[r, 0:128])
                nc.sync.dma_start(out=xq[d][:, 0:hq], in_=xq_d[r, 0:hq])
                nc.sync.dma_start(out=wq[d][:, 0:128], in_=wq_d[r, 0:128])
            for d in range(DT):
                r = slice(d * 128, (d + 1) * 128)
                if h1 < INNER:
                    nc.sync.dma_start(out=wv[d][:, h1:], in_=wv_d[r, h1:])
                nc.sync.dma_start(out=wk[d][:, 128:], in_=wk_d[r, 128:])
                if hq < TQ:
                    nc.sync.dma_start(out=xq[d][:, hq:], in_=xq_d[r, hq:])
                nc.sync.dma_start(out=wq[d][:, 128:], in_=wq_d[r, 128:])

            # ---------- production tasks ----------
            def v_task(t, off, w):
                def run():
                    ps = ps_s.tile([128, 1024], F32, tag="sp", name="prv")
                    for d in range(DT):
                        nc.tensor.matmul(
                            ps[:, :w],
                            lhsT=xkv[d][:, t * 128:(t + 1) * 128],
                            rhs=wv[d][:, off:off + w],
                            start=(d == 0), stop=(d == DT - 1),
                        )
                    h0, nh = off // 64, w // 64
                    dst = v_t[t].rearrange("p (h e) -> p h e", e=65)
                    nc.vector.tensor_copy(
                        dst[:, h0:h0 + nh, 0:64],
                        ps[:, :w].rearrange("p (h e) -> p h e", e=64),
                    )
                    nc.vector.memset(dst[:, h0:h0 + nh, 64:65], 1.0)
                return run

            def kq_task(kt, c, tb, w, wsrc, xsrc):
                def run():
                    ps = ps_s.tile([128, 1024], F32, tag="sp", name="prk")
                    for d in range(DT):
                        nc.tensor.matmul(
                            ps[:, :w],
                            lhsT=wsrc[d][:, c * 128:(c + 1) * 128],
                            rhs=xsrc[d][:, tb:tb + w],
                            start=(d == 0), stop=(d == DT - 1),
                        )
                    nc.vector.tensor_copy(kt[:, tb:tb + w], ps[:, :w])
                return run

            def kq_tasks(c):
                kt = kqrot.tile([128, TKV], BF16, tag="kT", name=f"kT{c}")
                qt = kqrot.tile([128, TQ], BF16, tag="qT", name=f"qT{c}")
                tasks = [kq_task(kt, c, tb, w, wk, xkv) for tb, w in fblocks(TKV)]
                tasks += [kq_task(qt, c, tb, w, wq, xq) for tb, w in fblocks(TQ)]
                return kt, qt, tasks

            # PE warmup: dependency-free matmuls on constant tiles run
            # during the initial DMA wave and un-throttle the HAM clock
            # before the first real production matmuls.
            wps = ps_s.tile([128, 1024], F32, tag="sp", name="warm")
            for _ in range(56):
                nc.tensor.matmul(
                    wps[0:128, 0:128], lhsT=ones_row, rhs=ones_row[:, 0:128],
                    start=True, stop=True,
                )
            nc.vector.tensor_copy(
                work.tile([128, 128], F32, tag="wsink", name="wsink"),
                wps[:, 0:128])

            # v columns are produced just-in-time by half: heads 0..H/2-1
            # (first 512-block) feed rounds 0..HP/2-1, the rest feed later
            # rounds. v[0],v[1] (first half) plus pair 0's kT/qT up front.
            VHALF = fblocks(INNER)
            # six v tiles up front: this PE work runs while the second DMA
            # wave streams, and round 0's per-chunk task load stays light
            # enough that ScalarE never starves.
            NVUP = min(8, TT)
            for t in range(NVUP):
                v_task(t, *VHALF[0])()
            kt_cur, qt_cur, tasks0 = kq_tasks(0)
            nkt = len(fblocks(TKV))
            # only the first kT/qT blocks up front; the rest just-in-time in
            # round 0 (block b of kT is first read at chunk jt = 4*b)
            rest0 = tasks0[1:nkt] + tasks0[nkt + 1:]
            tasks0[0]()
            tasks0[nkt]()

            pending = []
            # second-half v tiles (and the whole tail of VHALF splits) spread
            # across rounds 1..HP/2; they are only read from round HP/2 on.
            spares = []
            for off, w in VHALF[1:]:
                spares += [v_task(t, off, w) for t in range(TT)]

            # aoT stays UNNORMALIZED through the attention rounds; the
            # softmax denominators are stashed per head in den_all and the
            # normalization happens in stage 3 (one wide ACT reciprocal +
            # DMA broadcast + DVE mults, all off the round boundaries).
            den_all = persist.tile([2 * HP, TQ], F32, tag="den_all",
                                   name="den_all")

            def copy_piece(hp, sub, aot, ioff, iw):
                # engines can only address 32-aligned partition bases, so the
                # den row goes through a base-0 staging tile + SBUF-SBUF DMA.
                def run():
                    nc.vector.tensor_copy(
                        aoT[hp][sub * 64:sub * 64 + 64, ioff:ioff + iw],
                        aot[0:64, 0:iw])
                    stg = work.tile([1, 512], F32, tag="dstg", name="dstg")
                    nc.vector.tensor_copy(stg[:, :iw], aot[64:65, 0:iw])
                    h = 2 * hp + sub
                    nc.sync.dma_start(
                        out=den_all[h:h + 1, ioff:ioff + iw],
                        in_=stg[:, :iw])
                return run

            deferred = []
            late = None
            dramp = tc.alloc_tile_pool(name="dramp", bufs=1, space="DRAM")
            for hp in range(HP):
                if hp + 1 == HP:
                    # s1 (x / qkv weights) is dead once hp=7's kT/qT exist;
                    # reclaim it early so stage-3 tiles + wo prefetch can
                    # start during the last round.
                    s1.release()
                    late = tc.alloc_tile_pool(name="late", bufs=1)
                    wo = [late.tile([128, DIM], BF16, tag=f"wo{c}",
                                    name=f"wo{c}") for c in range(CT)]
                    for c in range(CT):
                        nc.sync.dma_start(
                            out=wo[c], in_=wo_d[c * 128:(c + 1) * 128, :])
                    recl = late.tile([2 * HP, TQ], F32, tag="recl",
                                     name="recl")
                    recb = late.tile([2 * HP, TQ], BF16, tag="recb16",
                                     name="recb16")
                    recd = dramp.tile([2 * HP, TQ], BF16, tag="recd",
                                      name="recd")
                    rbt = [late.tile([128, TQ], BF16, tag=f"rbt{c}",
                                     name=f"rbt{c}") for c in range(CT)]
                    aoTn = [late.tile([128, TQ], BF16, tag=f"an{c}",
                                      name=f"an{c}") for c in range(CT)]

                    def den_batch(rlo, rhi, clo, chi):
                        # reciprocal of den rows [rlo:rhi] + rb broadcast and
                        # aoT normalize for c-tiles [clo:chi]. ACT ops always
                        # start at partition 0 (engine partition alignment);
                        # only the DMA slices the fresh rows.
                        def run():
                            nc.scalar.activation(recl[0:rhi],
                                                 den_all[0:rhi], AF.Ln)
                            nc.scalar.activation(recb[0:rhi],
                                                 recl[0:rhi],
                                                 AF.Exp, scale=-1.0)
                            nc.sync.dma_start(out=recd[rlo:rhi, :],
                                              in_=recb[rlo:rhi])
                            for c in range(clo, chi):
                                for half in range(2):
                                    rsl = recd[2 * c + half:
                                               2 * c + half + 1, :]
                                    src = bass.AP(
                                        tensor=rsl.tensor, offset=rsl.offset,
                                        ap=[[0, 64]] + list(rsl.ap)[1:])
                                    nc.sync.dma_start(
                                        out=rbt[c][64 * half:64 * half + 64,
                                                   :],
                                        in_=src)
                                nc.vector.tensor_mul(aoTn[c], aoT[c], rbt[c])
                        return run
                if hp + 1 < HP:
                    kt_nxt, qt_nxt, nxt = kq_tasks(hp + 1)
                    pending.extend(nxt)
                if 1 <= hp < max(2, HP // 2):
                    nsp = (len(spares) + max(2, HP // 2) - 1 - hp) // (
                        max(2, HP // 2) - hp)
                    pending.extend(spares[:nsp])
                    spares = spares[nsp:]
                h0, h1 = 2 * hp, 2 * hp + 1
                for i, (ioff, iw) in enumerate(IH):
                    ao0 = ps_ao.tile([65, 512], F32, tag="ao", name="ao")
                    ao1 = ps_ao.tile([65, 512], F32, tag="ao", name="ao")
                    # software pipeline: PV consumes p several chunks behind
                    # the scores/exp front so the PE never waits on a fresh
                    # exp (and the round-boundary norm ACT ops hide too).
                    LAG = 5
                    pq = []
                    for jt in range(TT + LAG):
                        if jt < TT:
                            # scores+exp first: ScalarE gets its input at
                            # chunk start, production tasks follow in its
                            # shadow
                            sp = ps_s.tile([128, 1024], F32, tag="sp", name="sp")
                            nc.tensor.matmul(
                                sp[:, 0:iw],
                                lhsT=kt_cur[0:64, jt * 128:(jt + 1) * 128],
                                rhs=qt_cur[0:64, ioff:ioff + iw],
                                start=True, stop=True, tile_position=(0, 0),
                            )
                            nc.tensor.matmul(
                                sp[:, iw:2 * iw],
                                lhsT=kt_cur[64:128, jt * 128:(jt + 1) * 128],
                                rhs=qt_cur[64:128, ioff:ioff + iw],
                                start=True, stop=True, tile_position=(64, 0),
                            )
                            p = ppool.tile([128, 1024], BF16, tag="p", name="p")
                            nc.scalar.activation(p[:, 0:2 * iw], sp[:, 0:2 * iw],
                                                 AF.Exp, scale=scale)
                            pq.append(p)
                            if hp == 0 and i == 0:
                                if jt in (0, 2, 4, 8) and rest0:
                                    rest0.pop(0)()
                                if jt + NVUP < TT:
                                    v_task(jt + NVUP, *VHALF[0])()
                            elif jt % 3 == 1 and pending:
                                pending.pop(0)()
                        if jt >= LAG:
                            pjt = jt - LAG
                            p = pq.pop(0)
                            first, last = (pjt == 0), (pjt == TT - 1)
                            nc.tensor.matmul(
                                ao0[0:65, 0:iw],
                                lhsT=v_t[pjt][:, h0 * 65:h0 * 65 + 65],
                                rhs=p[:, 0:iw],
                                start=first, stop=last,
                            )
                            nc.tensor.matmul(
                                ao1[0:65, 0:iw],
                                lhsT=v_t[pjt][:, h1 * 65:h1 * 65 + 65],
                                rhs=p[:, iw:2 * iw],
                                start=first, stop=last,
                            )
                        if jt in (0, 3) and deferred:
                            deferred.pop(0)()
                        if hp + 1 == HP and i == 1 and jt == TT:
                            # ScalarE goes idle in the final round's tail:
                            # run the reciprocal chain for heads 0..13 here.
                            den_batch(0, 2 * HP - 2, 0, CT - 1)()
                    deferred.append(copy_piece(hp, 0, ao0, ioff, iw))
                    deferred.append(copy_piece(hp, 1, ao1, ioff, iw))
                if hp + 1 < HP:
                    while pending:
                        pending.pop(0)()
                    kt_cur, qt_cur = kt_nxt, qt_nxt
            while deferred:
                deferred.pop(0)()
            # last two heads' reciprocal chain (their den rows finalize only
            # after the final round's copies)
            den_batch(2 * HP - 2, 2 * HP, CT - 1, CT)()
            ps_ao.release()

            # ---- stage 3: out-projection + bias + LayerNorm ----
            yout = tc.alloc_tile_pool(name="yout", bufs=2)
            for t in range(NQT):
                yps = ps_s.tile([128, DIM], F32, tag="sp", name="yps")
                for c in range(CT):
                    for off, w in fblocks(DIM):
                        nc.tensor.matmul(
                            yps[:, off:off + w],
                            lhsT=aoTn[c][:, t * 128:(t + 1) * 128],
                            rhs=wo[c][:, off:off + w],
                            start=(c == 0), stop=False,
                        )
                for off, w in fblocks(DIM):
                    nc.tensor.matmul(
                        yps[:, off:off + w],
                        lhsT=ones_row,
                        rhs=bo_sb[:, off:off + w],
                        start=False, stop=True,
                    )
                ng = (DIM + 511) // 512
                st = work.tile([128, ng, 6], F32, tag="bnst", name="bnst")
                for g in range(ng):
                    gw = min(512, DIM - g * 512)
                    nc.vector.bn_stats(st[:, g, :], yps[:, g * 512:g * 512 + gw])
                mv = work.tile([128, 2], F32, tag="mv", name="mv")
                nc.vector.bn_aggr(mv, st)
                # rstd = sqrt(1/(var+eps)): DVE reciprocal + ACT sqrt keeps
                # the heavy per-element work off ScalarE.
                ve = work.tile([128, 1], F32, tag="lnv", name="ve")
                nc.vector.tensor_scalar(ve, mv[:, 1:2], scalar1=EPS,
                                        scalar2=None, op0=ALU.add)
                vr = work.tile([128, 1], F32, tag="vr", name="vr")
                nc.vector.reciprocal(vr, ve)
                rstd = work.tile([128, 1], F32, tag="rstd", name="rstd")
                nc.scalar.activation(rstd, vr, AF.Sqrt)
                nmr = work.tile([128, 1], F32, tag="nmr", name="nmr")
                nc.vector.tensor_scalar(
                    nmr, mv[:, 0:1], scalar1=rstd, scalar2=-1.0,
                    op0=ALU.mult, op1=ALU.mult,
                )
                # (y - mu)*rstd on ScalarE (idle in this phase): Identity
                # with per-partition scale/bias APs.
                yA = yout.tile([128, DIM], F32, tag="yA", name="yA")
                nc.scalar.activation(yA, yps, AF.Identity, bias=nmr,
                                     scale=rstd)
                yn = yout.tile([128, DIM], F32, tag="yn", name="yn")
                nc.vector.tensor_mul(yn, yA, gamma_b)
                yn2 = yout.tile([128, DIM], F32, tag="yn2", name="yn2")
                # beta add on GpSimd mid-stage (spreads load), DVE for the
                # last blocks (GpSimd's 2.3us latency would extend the tail)
                eng = nc.gpsimd if t < NQT - 2 else nc.vector
                eng.tensor_add(yn2, yn, beta_b)
                nc.sync.dma_start(out=y_d[t * 128:(t + 1) * 128, :], in_=yn2)
            yout.release()
            late.release()

    return nc


def host_inputs(x_b, r, w_qkv, w_out, b_out, ln_gamma, ln_beta, TQ):
    """Build one core's input map from full fp32 arrays. x_b: [T, DIM]."""
    bf = ml_dtypes.bfloat16
    DIM = x_b.shape[1]
    INNER = w_qkv.shape[1] // 3
    xT = np.ascontiguousarray(x_b.T).astype(bf)
    xTq = np.ascontiguousarray(x_b[r * TQ:(r + 1) * TQ].T).astype(bf)
    return {
        "xT_kv": xT,
        "xT_q": xTq,
        "w_q": np.ascontiguousarray(w_qkv[:, :INNER]).astype(bf),
        "w_k": np.ascontiguousarray(w_qkv[:, INNER:2 * INNER]).astype(bf),
        "w_v": np.ascontiguousarray(w_qkv[:, 2 * INNER:]).astype(bf),
        "w_out": np.ascontiguousarray(w_out).astype(bf),
        "b_out": np.ascontiguousarray(b_out[None, :]).astype(bf),
        "ln_gamma": np.ascontiguousarray(ln_gamma).astype(np.float32),
        "ln_beta": np.ascontiguousarray(ln_beta).astype(np.float32),
    }


_NC_CACHE = {}


def _get_nc():
    if "nc" not in _NC_CACHE:
        nc = build_nc()
        split_excess_waits(nc)
        _NC_CACHE["nc"] = nc
    return _NC_CACHE["nc"]


def kernel(x, w_qkv, w_out, b_out, ln_gamma, ln_beta):
    x = np.asarray(x, dtype=np.float32)
    w_qkv = np.asarray(w_qkv, dtype=np.float32)
    w_out = np.asarray(w_out, dtype=np.float32)
    b_out = np.asarray(b_out, dtype=np.float32)
    ln_gamma = np.asarray(ln_gamma, dtype=np.float32)
    ln_beta = np.asarray(ln_beta, dtype=np.float32)
    B, N, DIM = x.shape
    TQ = N // 2
    nc = _get_nc()
    in_maps = []
    for c in range(8):
        b, r = c // 2, c % 2
        in_maps.append(host_inputs(
            x[b], r, w_qkv, w_out, b_out, ln_gamma, ln_beta, TQ))
    res = run_bass_kernel_spmd(nc, in_maps, list(range(8)))
    out = np.empty((B, N, DIM), np.float32)
    for c in range(8):
        b, r = c // 2, c % 2
        out[b, r * TQ:(r + 1) * TQ] = res.results[c]["y"]
    return out

